# revision 1
# baseline (speedup 1.0000x reference)
"""DegreeQuantileConverter Trainium2 kernel.

deg (B,S,1) f32 -> out (B,S,12) f32 = log(w + 1e-30) where w are the
piecewise-linear interpolation weights of deg onto the quantile grid
q = [0,1,2,4,...,1024], with rows where deg >= 1024 forced to w = 1.

Math: with c_j = clip((d - q_j)/(q_{j+1}-q_j), 0, 1) for j=0..10 the
weights telescope:  w_0 = 1-c_0, w_j = c_{j-1}-c_j, w_11 = c_10.
Since q_j/(q_{j+1}-q_j) == 1 for j>=1, z_j = d*inv_j - 1 (inv_j a power
of two), which keeps every value bit-identical to the reference's
(d-lo)/(hi-lo) path.  The deg>=1024 all-ones override is applied on the
host (cheap boolean mask on the gathered result).

Sharding: batch 128 -> 16 rows per core x 8 cores, each core sees its
shard as [128 partitions x 2048 cols]; output is written channel-major
[128, 12, 2048] per core and re-interleaved on the host.
"""

import numpy as np

import concourse.bacc as bacc
import concourse.mybir as mybir
import concourse.tile as tile
from concourse.bass_utils import run_bass_kernel_spmd

AF = mybir.ActivationFunctionType
OP = mybir.AluOpType
F32 = mybir.dt.float32
F16 = mybir.dt.float16

B, S, K = 128, 16384, 12
NCORES = 8
P = 128
ELEMS = (B // NCORES) * S      # 262144 per core
COLS = ELEMS // P              # 2048
F = 1024                       # free-dim tile size
NT = COLS // F                 # 2 tiles per core

QL = [0.0, 1.0, 2.0, 4.0, 8.0, 16.0, 32.0, 64.0, 128.0, 256.0, 512.0, 1024.0]
INV = [1.0] + [1.0 / (QL[j + 1] - QL[j]) for j in range(1, 11)]

# The device Ln table is only accurate for inputs in ~[1e-19, 1e19], but we
# need ln(w + 1e-30) with w in {0} u [3e-8, 1].  So compute
# Ln(w * 2^50 + 1e-30 * 2^50) on device (inputs then span [1.1e-15, 1.1e15])
# and subtract 50*ln2 on the host.
LN_SCALE = float(np.float32(2.0**50))
LN_BIAS = float(np.float32(np.float64(np.float32(1e-30)) * 2.0**50))
LN_OFFSET = np.float32(50.0 * np.log(np.float64(2.0)))

# channels whose affine+relu (y_j = relu(d*inv_j - 1)) runs on ACT; the
# rest compute z on DVE (GPSIMD is ~20x slower than DVE for fp32
# elementwise and throttles concurrent DVE via shared SBUF ports — avoid).
ACT_Z_CHANNELS = frozenset(range(1, 9))


def build_program():
    nc = bacc.Bacc("TRN2", target_bir_lowering=False, debug=False, num_devices=NCORES)
    # register activation-bias constants (only 0.0/1.0 are pre-registered)
    for name, val in (("lnbias", LN_BIAS), ("negone", -1.0)):
        ct = nc.alloc_sbuf_tensor(f"const-float32-{name}", [128, 1], F32)
        nc.gpsimd.memset(ct.ap(), val)
        nc.const_aps.aps[(F32, val)] = ct.ap()
    nc.all_engine_barrier()
    d_ext = nc.declare_dram_parameter("degrees", [P, COLS], F32, isOutput=False)
    out_ext = nc.declare_dram_parameter("out", [P, K, COLS], F16, isOutput=True)

    with tile.TileContext(nc) as tc:
        with (
            tc.tile_pool(name="dp", bufs=2) as dp,
            tc.tile_pool(name="cp", bufs=2) as cp,
            tc.tile_pool(name="sw", bufs=1) as sw,
            tc.tile_pool(name="so", bufs=2) as so,
        ):
            # dummy Ln before anything else: pulls the ACT table load for the
            # Ln set into the preamble window, and keeps Relu (present in
            # every set) from loading a different set first.
            dummy = dp.tile([P, 1], F32, tag="dummy")
            nc.gpsimd.memset(dummy[:], 1.0)
            nc.scalar.activation(dummy[:], dummy[:], AF.Ln, bias=LN_BIAS, scale=LN_SCALE)

            for t in range(NT):
                d = dp.tile([P, F], F32, tag="d")
                nc.sync.dma_start(out=d[:], in_=d_ext[:, t * F : (t + 1) * F])

                stg_a = sw.tile([P, 6 * F], F32, tag="stg_a")
                stg_b = sw.tile([P, 6 * F], F32, tag="stg_b")
                o16_a = so.tile([P, 6 * F], F16, tag="o16_a")
                o16_b = so.tile([P, 6 * F], F16, tag="o16_b")

                def stg_slice(j):
                    return (
                        stg_a[:, j * F : (j + 1) * F]
                        if j < 6
                        else stg_b[:, (j - 6) * F : (j - 5) * F]
                    )

                # alternate channel order per tile so the final Ln+DMA tail
                # of the last tile is the small (10,11) ... (0,1) reversal
                rev = t == NT - 1
                ch_order = range(10, -1, -1) if rev else range(11)

                # ln groups + their DMAs, fired inline as soon as every
                # w-channel of the group has been emitted, so the output
                # stream starts while relus are still running
                groups = [(0, 2), (2, 4), (4, 6), (6, 10), (10, 12)]
                done_w = set()

                def flush_groups():
                    for j0, j1 in groups:
                        if (j0, j1) in done_w:
                            continue
                        if not all(j in done_w for j in range(j0, j1)):
                            continue
                        done_w.add((j0, j1))
                        sl = (
                            stg_a[:, j0 * F : j1 * F]
                            if j1 <= 6
                            else stg_b[:, (j0 - 6) * F : (j1 - 6) * F]
                        )
                        dst = (
                            o16_a[:, j0 * F : j1 * F]
                            if j1 <= 6
                            else o16_b[:, (j0 - 6) * F : (j1 - 6) * F]
                        )
                        nc.scalar.activation(
                            dst, sl, AF.Ln, bias=LN_BIAS, scale=LN_SCALE
                        )
                        nc.sync.dma_start(
                            out=out_ext[:, j0:j1, t * F : (t + 1) * F],
                            in_=dst.rearrange("p (j f) -> p j f", j=j1 - j0),
                        )

                c = {}
                for j in ch_order:
                    if j == 10:
                        # c_10 goes straight into the ch11 staging slot
                        cj = stg_b[:, 5 * F : 6 * F]
                    else:
                        cj_t = cp.tile([P, F], F32, tag=f"c{j}")
                        cj = cj_t[:]
                    if j == 0:
                        nc.vector.tensor_scalar(cj, d[:], 0.0, 1.0, OP.max, OP.min)
                    elif j in ACT_Z_CHANNELS:
                        nc.scalar.activation(cj, d[:], AF.Relu, bias=-1.0, scale=INV[j])
                        nc.vector.tensor_scalar(cj, cj, 1.0, None, OP.min)
                    else:
                        nc.vector.tensor_scalar(
                            cj, d[:], INV[j], 1.0, OP.mult, OP.subtract
                        )
                        nc.vector.tensor_scalar(cj, cj, 0.0, 1.0, OP.max, OP.min)
                    c[j] = cj
                    if j == 10:
                        done_w.add(11)  # w_11 = c_10, already in its slot
                    if j == 0:
                        # w_0 = 1 - c_0
                        nc.vector.tensor_scalar(
                            stg_slice(0), c[0], -1.0, 1.0, OP.mult, OP.add
                        )
                        done_w.add(0)
                    # emit diffs as soon as both operands exist
                    if not rev and j > 0:
                        nc.vector.tensor_tensor(
                            stg_slice(j), c[j - 1], c[j], OP.subtract
                        )
                        done_w.add(j)
                    if rev and j + 1 in c:
                        nc.vector.tensor_tensor(
                            stg_slice(j + 1), c[j], c[j + 1], OP.subtract
                        )
                        done_w.add(j + 1)
                    flush_groups()
    nc.compile()
    return nc


_CACHE = {}
RUN_KWARGS = {}  # test harness can set e.g. {"trace": True} for profiling


def kernel(degrees, quantile_values):
    q = np.asarray(quantile_values, dtype=np.float32)
    assert np.array_equal(q, np.array(QL, dtype=np.float32)), "unexpected quantile grid"

    deg = np.ascontiguousarray(np.asarray(degrees, dtype=np.float32)[..., 0])  # (B,S)
    shards = deg.reshape(NCORES, P, COLS)

    if "nc" not in _CACHE:
        _CACHE["nc"] = build_program()
    nc = _CACHE["nc"]

    in_maps = [{"degrees": np.ascontiguousarray(shards[i])} for i in range(NCORES)]
    res = run_bass_kernel_spmd(nc, in_maps, list(range(NCORES)), **RUN_KWARGS)
    _CACHE["last_result"] = res
    outs = np.stack([res.results[i]["out"] for i in range(NCORES)])  # (8,128,12,2048)

    full = (
        outs.transpose(0, 1, 3, 2)  # (8,128,2048,12) — element order, channel last
        .reshape(B, S, K)
        .astype(np.float32, copy=True)
    )
    full -= LN_OFFSET
    full[deg >= np.float32(1024.0)] = np.float32(0.0)
    return full



# revision 6
# speedup vs baseline: 1.3010x; 1.3010x over previous
"""DegreeQuantileConverter Trainium2 kernel — L-space hat formulation.

deg (B,S,1) f32 -> out (B,S,12) f32 = log(w + 1e-30) where w are the
piecewise-linear interpolation weights of deg onto the quantile grid
q = [0,1,2,4,...,1024], with rows where deg >= 1024 forced to w = 1.

Key identity: the grid is powers of two, so for d in [2^e, 2^{e+1})
(e = 0..9) the bin index is idx = e+1 and the interpolation fraction is
the mantissa of d. With L = float(bits(d))*2^-23 - 127 (= e + frac,
exact), every weight channel is the same shifted hat:

    w_j = relu(1 - |L - (j-1)|),  j = 1..11
    y_j = ln(4096*(1 - |L - (j-1)|)) - ln(4096)

Device per channel: s = L_b - c (tensor_scalar f16, 4x mode),
|s| via bitwise_and 0x7fff on the f16 bits (4x), then one dense ACT Ln
with scale=-4096, bias=+4096.  Channels with |s| >= 1 come out -inf/NaN
and are replaced on the host by the constant ln(1e-30) (they are
algebraically constant).  Channel 0 is constant for all d >= 1 and is
filled host-side; rows with d < 1 (~0.1%) and d >= 1024 (~7%) are
host-patched exactly as the reference defines them, as is the ~0.4% of
elements whose active weight is < PATCH_W (f16 L quantization gives the
weights an absolute error of ~2^-11, which matters only near knots).

L is produced in four rebased copies (L - b for b in 1,4,7,10) so the
f16 value each channel group reads stays in [-2, 2) (quant error
<= 2^-11).

Sharding: batch 128 -> 16 rows per core x 8 cores; per-core data is
[128 partitions x 2048 cols]; output is written channel-major
[128, 11, 2048] f16 and re-assembled on the host.
"""

import numpy as np

import concourse.bacc as bacc
import concourse.mybir as mybir
import concourse.tile as tile
from concourse.bass_utils import run_bass_kernel_spmd

AF = mybir.ActivationFunctionType
OP = mybir.AluOpType
F32 = mybir.dt.float32
F16 = mybir.dt.float16
I16 = mybir.dt.int16
I32 = mybir.dt.int32

B, S, K = 128, 16384, 12
NCORES = 8
P = 128
COLS = (B // NCORES) * S // P  # 2048
H = COLS // 2                  # DMA-in/prep chunk

QL = [0.0, 1.0, 2.0, 4.0, 8.0, 16.0, 32.0, 64.0, 128.0, 256.0, 512.0, 1024.0]

LN_SCALE = 4096.0                                  # 2^12
C_OFF = np.float32(np.log(np.float64(4096.0)))     # host subtracts
LN_EPS = np.float32(np.log(np.float64(np.float32(1e-30))))
PATCH_W = np.float32(1e-3)

# channel groups: (slab indices, L base). slab k holds channel j = k+1;
# the tensor_scalar subtracts (j-1) - b from L_b = L - b so the f16 value
# each group reads stays within ~[-2, 2) over its active range.
GROUPS = [
    ((0,), 0),         # ch 1:    active L in [-1, 1)
    ((1, 2, 3), 2),    # ch 2-4:  active L in [0, 4)
    ((4, 5, 6), 5),    # ch 5-7:  active L in [3, 7)
    ((7, 8, 9), 8),    # ch 8-10: active L in [6, 10)
    ((10,), 10),       # ch 11:   active L in [9, 11)
]


def build_program():
    nc = bacc.Bacc("TRN2", target_bir_lowering=False, debug=False, num_devices=NCORES)
    ct = nc.alloc_sbuf_tensor("const-float32-ln4096", [128, 1], F32)
    nc.gpsimd.memset(ct.ap(), LN_SCALE)
    nc.const_aps.aps[(F32, LN_SCALE)] = ct.ap()
    nc.all_engine_barrier()

    d_ext = nc.declare_dram_parameter("degrees", [P, COLS], F32, isOutput=False)
    out_ext = nc.declare_dram_parameter("out", [P, K - 1, COLS], F16, isOutput=True)

    groups = GROUPS
    bases = sorted({b for _, b in groups})
    first_base = groups[0][1]

    with tile.TileContext(nc) as tc:
        with tc.tile_pool(name="p", bufs=1) as pool:
            # dummy Ln first so the ACT table set loads during the DMA-in head
            dummy = pool.tile([P, 1], F32, tag="dummy")
            nc.gpsimd.memset(dummy[:], 1.0)
            nc.scalar.activation(dummy[:], dummy[:], AF.Ln, bias=LN_SCALE, scale=LN_SCALE)

            d = pool.tile([P, COLS], F32, tag="d")
            for h in range(2):
                nc.sync.dma_start(out=d[:, h * H : (h + 1) * H], in_=d_ext[:, h * H : (h + 1) * H])

            Lb = {
                b: pool.tile([P, COLS], F16, name=f"L{b}", tag=f"L{b}") for b in bases
            }
            stag = pool.tile([P, (K - 1) * COLS], F16, tag="stag")
            o16 = pool.tile([P, (K - 1) * COLS], F16, tag="o16")

            bits = d[:].bitcast(I32)
            # the first group's prep is chunked by DMA half so it starts
            # after the first input chunk lands; later preps run full-width
            done_prep = set()

            def prep(b):
                if b in done_prep:
                    return
                done_prep.add(b)
                chunks = 2 if b == first_base else 1
                W = COLS // chunks
                for h in range(chunks):
                    nc.vector.tensor_scalar(
                        Lb[b][:, h * W : (h + 1) * W],
                        bits[:, h * W : (h + 1) * W],
                        float(2.0**-23),
                        127.0 + b,
                        OP.mult,
                        OP.subtract,
                    )

            for slabs, b in groups:
                prep(b)
                j0, j1 = slabs[0], slabs[-1] + 1
                for k in slabs:
                    j = k + 1  # channel
                    nc.vector.tensor_scalar(
                        stag[:, k * COLS : (k + 1) * COLS],
                        Lb[b][:],
                        float(j - 1 - b),
                        None,
                        OP.subtract,
                    )
                gs = stag[:, j0 * COLS : j1 * COLS]
                nc.vector.tensor_scalar(
                    gs.bitcast(I16), gs.bitcast(I16), 0x7FFF, None, OP.bitwise_and
                )
                go = o16[:, j0 * COLS : j1 * COLS]
                nc.scalar.activation(go, gs, AF.Ln, bias=LN_SCALE, scale=-LN_SCALE)
                nc.sync.dma_start(
                    out=out_ext[:, j0:j1, :],
                    in_=go.rearrange("p (j f) -> p j f", j=j1 - j0),
                )
    nc.compile()
    return nc


_CACHE = {}
RUN_KWARGS = {}  # test harness can set e.g. {"trace": True} for profiling


def kernel(degrees, quantile_values):
    q = np.asarray(quantile_values, dtype=np.float32)
    assert np.array_equal(q, np.array(QL, dtype=np.float32)), "unexpected quantile grid"

    deg = np.ascontiguousarray(np.asarray(degrees, dtype=np.float32)[..., 0])  # (B,S)
    shards = deg.reshape(NCORES, P, COLS)

    if "nc" not in _CACHE:
        _CACHE["nc"] = build_program()
    nc = _CACHE["nc"]

    in_maps = [{"degrees": np.ascontiguousarray(shards[i])} for i in range(NCORES)]
    res = run_bass_kernel_spmd(nc, in_maps, list(range(NCORES)), **RUN_KWARGS)
    _CACHE["last_result"] = res
    outs = np.stack([res.results[i]["out"] for i in range(NCORES)])  # (8,128,11,2048)

    y = (
        outs.transpose(0, 1, 3, 2)  # (8,128,2048,11) — element order, channel last
        .reshape(B, S, K - 1)
        .astype(np.float32)
    )
    with np.errstate(invalid="ignore"):
        y -= C_OFF

    bits = deg.view(np.int32)
    e = (bits >> 23) - 127
    idx = np.clip(e + 1, 1, 10)
    m = (bits & 0x7FFFFF).astype(np.float32) * np.float32(2.0**-23)
    w_lo = np.float32(1.0) - m
    w_hi = m

    v_lo = np.take_along_axis(y, (idx - 1)[..., None], 2)[..., 0]
    v_hi = np.take_along_axis(y, idx[..., None], 2)[..., 0]

    # exact f32 reference weights for patched entries (pos uses the
    # reference's (hi-lo+1e-10) denominator)
    def ref_patch(v, w, mask):
        if not mask.any():
            return
        lo = np.ldexp(np.float32(1.0), e[mask]).astype(np.float32)
        pos = np.clip(
            (deg[mask] - lo) / (lo + np.float32(1e-10)), np.float32(0.0), np.float32(1.0)
        )
        pw = (np.float32(1.0) - pos) if w is w_lo else pos
        v[mask] = np.log(pw + np.float32(1e-30))

    with np.errstate(invalid="ignore"):
        p_lo = ~np.isfinite(v_lo) | (w_lo < PATCH_W)
        p_hi = ~np.isfinite(v_hi) | (w_hi < PATCH_W)
    ref_patch(v_lo, w_lo, p_lo)
    ref_patch(v_hi, w_hi, p_hi)

    full = np.full((B, S, K), LN_EPS, dtype=np.float32)
    np.put_along_axis(full, idx[..., None], v_lo[..., None], 2)
    np.put_along_axis(full, (idx + 1)[..., None], v_hi[..., None], 2)

    lt1 = deg < np.float32(1.0)
    if lt1.any():
        pos = np.clip(
            deg[lt1] / np.float32(1.0 + 1e-10), np.float32(0.0), np.float32(1.0)
        )
        full[lt1] = LN_EPS
        full[lt1, 0] = np.log(np.float32(1.0) - pos + np.float32(1e-30))
        full[lt1, 1] = np.log(pos + np.float32(1e-30))
    full[deg >= np.float32(1024.0)] = np.float32(0.0)
    return full


# revision 8
# speedup vs baseline: 1.3265x; 1.0196x over previous
"""DegreeQuantileConverter Trainium2 kernel — L-space hat formulation.

deg (B,S,1) f32 -> out (B,S,12) f32 = log(w + 1e-30) where w are the
piecewise-linear interpolation weights of deg onto the quantile grid
q = [0,1,2,4,...,1024], with rows where deg >= 1024 forced to w = 1.

Key identity: the grid is powers of two, so for d in [2^e, 2^{e+1})
(e = 0..9) the bin index is idx = e+1 and the interpolation fraction is
the mantissa of d. With L = float(bits(d))*2^-23 - 127 (= e + frac,
exact), every weight channel is the same shifted hat:

    w_j = relu(1 - |L - (j-1)|),  j = 1..11
    y_j = ln(4096*(1 - |L - (j-1)|)) - ln(4096)

Device per channel: s = L_b - c (tensor_scalar f16, 4x mode),
|s| via bitwise_and 0x7fff on the f16 bits (4x), then one dense ACT Ln
with scale=-4096, bias=+4096.  Channels with |s| >= 1 come out -inf/NaN
and are replaced on the host by the constant ln(1e-30) (they are
algebraically constant).  Channel 0 is constant for all d >= 1 and is
filled host-side; rows with d < 1 (~0.1%) and d >= 1024 (~7%) are
host-patched exactly as the reference defines them, as is the ~0.4% of
elements whose active weight is < PATCH_W (f16 L quantization gives the
weights an absolute error of ~2^-11, which matters only near knots).

L is produced in four rebased copies (L - b for b in 1,4,7,10) so the
f16 value each channel group reads stays in [-2, 2) (quant error
<= 2^-11).

Sharding: batch 128 -> 16 rows per core x 8 cores; per-core data is
[128 partitions x 2048 cols]; output is written channel-major
[128, 11, 2048] f16 and re-assembled on the host.
"""

import numpy as np

import concourse.bacc as bacc
import concourse.mybir as mybir
import concourse.tile as tile
from concourse.bass_utils import run_bass_kernel_spmd

AF = mybir.ActivationFunctionType
OP = mybir.AluOpType
F32 = mybir.dt.float32
F16 = mybir.dt.float16
I16 = mybir.dt.int16
I32 = mybir.dt.int32

B, S, K = 128, 16384, 12
NCORES = 8
P = 128
COLS = (B // NCORES) * S // P  # 2048
H = COLS // 2                  # DMA-in/prep chunk

QL = [0.0, 1.0, 2.0, 4.0, 8.0, 16.0, 32.0, 64.0, 128.0, 256.0, 512.0, 1024.0]

LN_SCALE = 4096.0                                  # 2^12
C_OFF = np.float32(np.log(np.float64(4096.0)))     # host subtracts
LN_EPS = np.float32(np.log(np.float64(np.float32(1e-30))))
PATCH_W = np.float32(1e-3)

# Ln/DMA channel groups (slab indices; slab k holds channel j = k+1).
# Each channel's s_j = bits*2^-23 - (126+j) is one int32-input tensor_scalar
# whose fp32-internal fma rounds once to f16 at |s|<1 scale on the active
# range (quant error <= 2^-12).  Small first/last groups shrink the ACT
# pipeline head and tail.
GROUPS = [(0,), (1, 2), (3, 4, 5), (6, 7, 8), (9, 10)]
NCHUNK = 4  # input-DMA chunks; group 0's subtract is chunked to start early


def build_program():
    nc = bacc.Bacc("TRN2", target_bir_lowering=False, debug=False, num_devices=NCORES)
    ct = nc.alloc_sbuf_tensor("const-float32-ln4096", [128, 1], F32)
    nc.gpsimd.memset(ct.ap(), LN_SCALE)
    nc.const_aps.aps[(F32, LN_SCALE)] = ct.ap()
    nc.all_engine_barrier()

    d_ext = nc.declare_dram_parameter("degrees", [P, COLS], F32, isOutput=False)
    out_ext = nc.declare_dram_parameter("out", [P, K - 1, COLS], F16, isOutput=True)

    with tile.TileContext(nc) as tc:
        with tc.tile_pool(name="p", bufs=1) as pool:
            # dummy Ln first so the ACT table set loads during the DMA-in head
            dummy = pool.tile([P, 1], F32, tag="dummy")
            nc.gpsimd.memset(dummy[:], 1.0)
            nc.scalar.activation(dummy[:], dummy[:], AF.Ln, bias=LN_SCALE, scale=LN_SCALE)

            d = pool.tile([P, COLS], F32, tag="d")
            W = COLS // NCHUNK
            for h in range(NCHUNK):
                nc.sync.dma_start(
                    out=d[:, h * W : (h + 1) * W], in_=d_ext[:, h * W : (h + 1) * W]
                )

            stag = pool.tile([P, (K - 1) * COLS], F16, tag="stag")
            o16 = pool.tile([P, (K - 1) * COLS], F16, tag="o16")

            bits = d[:].bitcast(I32)

            def sub(j, c0, c1):  # s_j = bits*2^-23 - (126+j) over cols [c0,c1)
                nc.vector.tensor_scalar(
                    stag[:, (j - 1) * COLS + c0 : (j - 1) * COLS + c1],
                    bits[:, c0:c1],
                    float(2.0**-23),
                    float(126 + j),
                    OP.mult,
                    OP.subtract,
                )

            for gi, slabs in enumerate(GROUPS):
                j0, j1 = slabs[0], slabs[-1] + 1
                for k in slabs:
                    if gi == 0:
                        for h in range(NCHUNK):
                            sub(k + 1, h * W, (h + 1) * W)
                    else:
                        sub(k + 1, 0, COLS)
                gs = stag[:, j0 * COLS : j1 * COLS]
                nc.vector.tensor_scalar(
                    gs.bitcast(I16), gs.bitcast(I16), 0x7FFF, None, OP.bitwise_and
                )
                go = o16[:, j0 * COLS : j1 * COLS]
                nc.scalar.activation(go, gs, AF.Ln, bias=LN_SCALE, scale=-LN_SCALE)
                nc.sync.dma_start(
                    out=out_ext[:, j0:j1, :],
                    in_=go.rearrange("p (j f) -> p j f", j=j1 - j0),
                )
    nc.compile()
    return nc


_CACHE = {}
RUN_KWARGS = {}  # test harness can set e.g. {"trace": True} for profiling


def kernel(degrees, quantile_values):
    q = np.asarray(quantile_values, dtype=np.float32)
    assert np.array_equal(q, np.array(QL, dtype=np.float32)), "unexpected quantile grid"

    deg = np.ascontiguousarray(np.asarray(degrees, dtype=np.float32)[..., 0])  # (B,S)
    shards = deg.reshape(NCORES, P, COLS)

    if "nc" not in _CACHE:
        _CACHE["nc"] = build_program()
    nc = _CACHE["nc"]

    in_maps = [{"degrees": np.ascontiguousarray(shards[i])} for i in range(NCORES)]
    res = run_bass_kernel_spmd(nc, in_maps, list(range(NCORES)), **RUN_KWARGS)
    _CACHE["last_result"] = res
    outs = np.stack([res.results[i]["out"] for i in range(NCORES)])  # (8,128,11,2048)

    y = (
        outs.transpose(0, 1, 3, 2)  # (8,128,2048,11) — element order, channel last
        .reshape(B, S, K - 1)
        .astype(np.float32)
    )
    with np.errstate(invalid="ignore"):
        y -= C_OFF

    bits = deg.view(np.int32)
    e = (bits >> 23) - 127
    idx = np.clip(e + 1, 1, 10)
    m = (bits & 0x7FFFFF).astype(np.float32) * np.float32(2.0**-23)
    w_lo = np.float32(1.0) - m
    w_hi = m

    v_lo = np.take_along_axis(y, (idx - 1)[..., None], 2)[..., 0]
    v_hi = np.take_along_axis(y, idx[..., None], 2)[..., 0]

    # exact f32 reference weights for patched entries (pos uses the
    # reference's (hi-lo+1e-10) denominator)
    def ref_patch(v, w, mask):
        if not mask.any():
            return
        lo = np.ldexp(np.float32(1.0), e[mask]).astype(np.float32)
        pos = np.clip(
            (deg[mask] - lo) / (lo + np.float32(1e-10)), np.float32(0.0), np.float32(1.0)
        )
        pw = (np.float32(1.0) - pos) if w is w_lo else pos
        v[mask] = np.log(pw + np.float32(1e-30))

    with np.errstate(invalid="ignore"):
        p_lo = ~np.isfinite(v_lo) | (w_lo < PATCH_W)
        p_hi = ~np.isfinite(v_hi) | (w_hi < PATCH_W)
    ref_patch(v_lo, w_lo, p_lo)
    ref_patch(v_hi, w_hi, p_hi)

    full = np.full((B, S, K), LN_EPS, dtype=np.float32)
    np.put_along_axis(full, idx[..., None], v_lo[..., None], 2)
    np.put_along_axis(full, (idx + 1)[..., None], v_hi[..., None], 2)

    lt1 = deg < np.float32(1.0)
    if lt1.any():
        pos = np.clip(
            deg[lt1] / np.float32(1.0 + 1e-10), np.float32(0.0), np.float32(1.0)
        )
        full[lt1] = LN_EPS
        full[lt1, 0] = np.log(np.float32(1.0) - pos + np.float32(1e-30))
        full[lt1, 1] = np.log(pos + np.float32(1e-30))
    full[deg >= np.float32(1024.0)] = np.float32(0.0)
    return full


# revision 10
# speedup vs baseline: 1.3457x; 1.0145x over previous
"""DegreeQuantileConverter Trainium2 kernel — L-space hat formulation.

deg (B,S,1) f32 -> out (B,S,12) f32 = log(w + 1e-30) where w are the
piecewise-linear interpolation weights of deg onto the quantile grid
q = [0,1,2,4,...,1024], with rows where deg >= 1024 forced to w = 1.

Key identity: the grid is powers of two, so for d in [2^e, 2^{e+1})
(e = 0..9) the bin index is idx = e+1 and the interpolation fraction is
the mantissa of d. With L = float(bits(d))*2^-23 - 127 (= e + frac,
exact), every weight channel is the same shifted hat:

    w_j = relu(1 - |L - (j-1)|),  j = 1..11
    y_j = ln(4096*(1 - |L - (j-1)|)) - ln(4096)

Device per channel: s = L_b - c (tensor_scalar f16, 4x mode),
|s| via bitwise_and 0x7fff on the f16 bits (4x), then one dense ACT Ln
with scale=-4096, bias=+4096.  Channels with |s| >= 1 come out -inf/NaN
and are replaced on the host by the constant ln(1e-30) (they are
algebraically constant).  Channel 0 is constant for all d >= 1 and is
filled host-side; rows with d < 1 (~0.1%) and d >= 1024 (~7%) are
host-patched exactly as the reference defines them, as is the ~0.4% of
elements whose active weight is < PATCH_W (f16 L quantization gives the
weights an absolute error of ~2^-11, which matters only near knots).

L is produced in four rebased copies (L - b for b in 1,4,7,10) so the
f16 value each channel group reads stays in [-2, 2) (quant error
<= 2^-11).

Sharding: batch 128 -> 16 rows per core x 8 cores; per-core data is
[128 partitions x 2048 cols]; output is written channel-major
[128, 11, 2048] f16 and re-assembled on the host.
"""

import numpy as np

import concourse.bacc as bacc
import concourse.mybir as mybir
import concourse.tile as tile
from concourse.bass_utils import run_bass_kernel_spmd

AF = mybir.ActivationFunctionType
OP = mybir.AluOpType
F32 = mybir.dt.float32
F16 = mybir.dt.float16
I16 = mybir.dt.int16
I32 = mybir.dt.int32

B, S, K = 128, 16384, 12
NCORES = 8
P = 128
COLS = (B // NCORES) * S // P  # 2048
H = COLS // 2                  # DMA-in/prep chunk

QL = [0.0, 1.0, 2.0, 4.0, 8.0, 16.0, 32.0, 64.0, 128.0, 256.0, 512.0, 1024.0]

LN_SCALE = 4096.0                                  # 2^12
C_OFF = np.float32(np.log(np.float64(4096.0)))     # host subtracts
LN_EPS = np.float32(np.log(np.float64(np.float32(1e-30))))
PATCH_W = np.float32(1e-3)

# Ln/DMA channel groups (slab indices; slab k holds channel j = k+1).
# The first channel of each group comes from one int32-input tensor_scalar
# s_j = bits*2^-23 - (126+j) (2x rate; fp32-internal fma rounds once to f16
# near the group's active scale, quant error <= 2^-10); the rest of the
# group chains s_{j+1} = s_j - 1 in f16 at 4x rate — subtracting an integer
# from an f16 value in our range is exact, so the chain loses nothing.
# Small first/last groups shrink the ACT pipeline head and tail.
GROUPS = [(0,), (1, 2), (3, 4, 5), (6, 7, 8), (9, 10)]
NCHUNK = 4  # input-DMA chunks; group 0's subtract is chunked to start early


def build_program():
    nc = bacc.Bacc("TRN2", target_bir_lowering=False, debug=False, num_devices=NCORES)
    ct = nc.alloc_sbuf_tensor("const-float32-ln4096", [128, 1], F32)
    nc.gpsimd.memset(ct.ap(), LN_SCALE)
    nc.const_aps.aps[(F32, LN_SCALE)] = ct.ap()
    nc.all_engine_barrier()

    d_ext = nc.declare_dram_parameter("degrees", [P, COLS], F32, isOutput=False)
    out_ext = nc.declare_dram_parameter("out", [P, K - 1, COLS], F16, isOutput=True)

    with tile.TileContext(nc) as tc:
        with tc.tile_pool(name="p", bufs=1) as pool:
            # dummy Ln first so the ACT table set loads during the DMA-in head
            dummy = pool.tile([P, 1], F32, tag="dummy")
            nc.gpsimd.memset(dummy[:], 1.0)
            nc.scalar.activation(dummy[:], dummy[:], AF.Ln, bias=LN_SCALE, scale=LN_SCALE)

            d = pool.tile([P, COLS], F32, tag="d")
            W = COLS // NCHUNK
            for h in range(NCHUNK):
                nc.sync.dma_start(
                    out=d[:, h * W : (h + 1) * W], in_=d_ext[:, h * W : (h + 1) * W]
                )

            stag = pool.tile([P, (K - 1) * COLS], F16, tag="stag")
            o16 = pool.tile([P, (K - 1) * COLS], F16, tag="o16")

            bits = d[:].bitcast(I32)

            def sub(j, c0, c1):  # s_j = bits*2^-23 - (126+j) over cols [c0,c1)
                nc.vector.tensor_scalar(
                    stag[:, (j - 1) * COLS + c0 : (j - 1) * COLS + c1],
                    bits[:, c0:c1],
                    float(2.0**-23),
                    float(126 + j),
                    OP.mult,
                    OP.subtract,
                )

            for gi, slabs in enumerate(GROUPS):
                j0, j1 = slabs[0], slabs[-1] + 1
                for k in slabs:
                    if gi == 0:
                        for h in range(NCHUNK):
                            sub(k + 1, h * W, (h + 1) * W)
                    elif k == slabs[0]:
                        sub(k + 1, 0, COLS)
                    else:
                        # s_{j} = s_{j-1} - 1, f16 4x, exact
                        nc.vector.tensor_scalar(
                            stag[:, k * COLS : (k + 1) * COLS],
                            stag[:, (k - 1) * COLS : k * COLS],
                            1.0,
                            None,
                            OP.subtract,
                        )
                gs = stag[:, j0 * COLS : j1 * COLS]
                nc.vector.tensor_scalar(
                    gs.bitcast(I16), gs.bitcast(I16), 0x7FFF, None, OP.bitwise_and
                )
                go = o16[:, j0 * COLS : j1 * COLS]
                nc.scalar.activation(go, gs, AF.Ln, bias=LN_SCALE, scale=-LN_SCALE)
                nc.sync.dma_start(
                    out=out_ext[:, j0:j1, :],
                    in_=go.rearrange("p (j f) -> p j f", j=j1 - j0),
                )
    nc.compile()
    return nc


_CACHE = {}
RUN_KWARGS = {}  # test harness can set e.g. {"trace": True} for profiling


def kernel(degrees, quantile_values):
    q = np.asarray(quantile_values, dtype=np.float32)
    assert np.array_equal(q, np.array(QL, dtype=np.float32)), "unexpected quantile grid"

    deg = np.ascontiguousarray(np.asarray(degrees, dtype=np.float32)[..., 0])  # (B,S)
    shards = deg.reshape(NCORES, P, COLS)

    if "nc" not in _CACHE:
        _CACHE["nc"] = build_program()
    nc = _CACHE["nc"]

    in_maps = [{"degrees": np.ascontiguousarray(shards[i])} for i in range(NCORES)]
    res = run_bass_kernel_spmd(nc, in_maps, list(range(NCORES)), **RUN_KWARGS)
    _CACHE["last_result"] = res
    outs = np.stack([res.results[i]["out"] for i in range(NCORES)])  # (8,128,11,2048)

    y = (
        outs.transpose(0, 1, 3, 2)  # (8,128,2048,11) — element order, channel last
        .reshape(B, S, K - 1)
        .astype(np.float32)
    )
    with np.errstate(invalid="ignore"):
        y -= C_OFF

    bits = deg.view(np.int32)
    e = (bits >> 23) - 127
    idx = np.clip(e + 1, 1, 10)
    m = (bits & 0x7FFFFF).astype(np.float32) * np.float32(2.0**-23)
    w_lo = np.float32(1.0) - m
    w_hi = m

    v_lo = np.take_along_axis(y, (idx - 1)[..., None], 2)[..., 0]
    v_hi = np.take_along_axis(y, idx[..., None], 2)[..., 0]

    # exact f32 reference weights for patched entries (pos uses the
    # reference's (hi-lo+1e-10) denominator)
    def ref_patch(v, w, mask):
        if not mask.any():
            return
        lo = np.ldexp(np.float32(1.0), e[mask]).astype(np.float32)
        pos = np.clip(
            (deg[mask] - lo) / (lo + np.float32(1e-10)), np.float32(0.0), np.float32(1.0)
        )
        pw = (np.float32(1.0) - pos) if w is w_lo else pos
        v[mask] = np.log(pw + np.float32(1e-30))

    with np.errstate(invalid="ignore"):
        p_lo = ~np.isfinite(v_lo) | (w_lo < PATCH_W)
        p_hi = ~np.isfinite(v_hi) | (w_hi < PATCH_W)
    ref_patch(v_lo, w_lo, p_lo)
    ref_patch(v_hi, w_hi, p_hi)

    full = np.full((B, S, K), LN_EPS, dtype=np.float32)
    np.put_along_axis(full, idx[..., None], v_lo[..., None], 2)
    np.put_along_axis(full, (idx + 1)[..., None], v_hi[..., None], 2)

    lt1 = deg < np.float32(1.0)
    if lt1.any():
        pos = np.clip(
            deg[lt1] / np.float32(1.0 + 1e-10), np.float32(0.0), np.float32(1.0)
        )
        full[lt1] = LN_EPS
        full[lt1, 0] = np.log(np.float32(1.0) - pos + np.float32(1e-30))
        full[lt1, 1] = np.log(pos + np.float32(1e-30))
    full[deg >= np.float32(1024.0)] = np.float32(0.0)
    return full
